# revision 5
# baseline (speedup 1.0000x reference)
"""Multi-head attention on 8 Trainium2 NeuronCores — v3 (woven pipeline).

Problem: x[4, 2048, 1024], 16 heads x 64 dim.
  qkv = x @ w_qkv; attn = softmax(q k^T / 8); out = (attn v) @ w_out + b_out

Sharding: 8 cores = 4 batches x 2 head-groups (8 heads each); host sums
the two partial out-projections per batch and adds the bias.

Engine model (from the cost model + TimelineSim): per attention cell
(pair, i-chunk, j-tile) the exp on ScalarE costs ~1038 ns vs ~853 ns of
TensorE work, so the 256-cell phase 2 is Activation-bound while TensorE
idles ~185 ns/cell; phases 1/3 are TensorE-only while ScalarE idles.
Engines execute their queues in order, so overlap must be programmed in
EMIT order. v3 therefore weaves one global instruction stream:

  - cells emit as [scores(n+1) | av(n)] (1-cell software pipeline) so the
    exp chain on ScalarE never waits on TensorE;
  - a slack-credit weaver inserts "filler" TensorE work between cells:
    this rep's out-projection groups and THE NEXT REP's phase-1 chunks
    (cross-rep pipelining; cpool bufs=2 double-buffers qkT/v across reps);
  - all inputs/SBUF tensors bf16 (PSUM fp32), halving DMA and SBUF;
  - one [128, 2*ICH] PSUM accumulator rotates across the 16 (pair, ic)
    streams; two forced fillers after each normalize cover its WAR gap.
"""

from collections import deque

import numpy as np

import concourse.bacc as bacc
import concourse.mybir as mybir
import concourse.tile as tile
from concourse.bass_utils import run_bass_kernel_spmd

F32 = mybir.dt.float32
BF16 = mybir.dt.bfloat16
F8 = mybir.dt.float8e4
PM = mybir.MatmulPerfMode.DoubleRow
AF = mybir.ActivationFunctionType

B = 4          # batch
N = 2048       # sequence
DM = 1024      # model dim
NH = 16        # heads
DH = 64        # head dim
G = 2          # head groups (cores per batch)
HPC = NH // G  # heads per core = 8
CW = DH * HPC  # per-core qkv column width = 512

NCH = 256      # phase-1 x^T column chunk
ICH = 512      # phase-2 i (query) chunk (per head; a pair shares [128, 2*ICH])

KT = DM // 128      # 8 contraction tiles over d
MT = 2 * CW // 128  # 8 c-tiles for q|k
NJT = N // 128      # 16 j tiles
NIC = N // ICH      # 4 i chunks

SLACK = 398.0       # ns of TensorE slack banked per cell for filler work
CREDIT_CAP = 900.0  # caps filler bursts that would starve the exp chain


def build_nc(reps=1):
    nc = bacc.Bacc(None, target_bir_lowering=False, debug=False)

    xT = nc.declare_dram_parameter("xT", [N // NCH, 128, KT * NCH], BF16,
                                   isOutput=False)
    wqk = nc.declare_dram_parameter("wqk", [DM, 2 * CW], BF16, isOutput=False)
    wv = nc.declare_dram_parameter("wv", [DM, CW], BF16, isOutput=False)
    wo = nc.declare_dram_parameter("wo", [CW, DM], BF16, isOutput=False)
    out = nc.declare_dram_parameter("out", [N, DM], F32, isOutput=True)

    with tile.TileContext(nc) as tc:
        with (
            # cross-rep double buffering for the phase-1 products
            tc.tile_pool(name="cpool", bufs=2) as cpool,
            # 8 PSUM banks: "s" 2x[128,1024] scores, "av" 1x[128,1024]
            # attention accumulator, "p1" 2x[128,512] projections
            tc.tile_pool(name="psA", bufs=2, space="PSUM") as psA,
            tc.tile_pool(name="psB", bufs=1, space="PSUM") as psB,
            tc.tile_pool(name="psC", bufs=2, space="PSUM") as psC,
            tc.tile_pool(name="epool", bufs=5) as epool,
            tc.tile_pool(name="wpool", bufs=1) as wpool,
            tc.tile_pool(name="lpool", bufs=2) as lpool,
            tc.tile_pool(name="xpool", bufs=2) as xpool,
            # bufs=1: each (m, half) stage is fully repacked before the next
            # half (or rep) rewrites it
            tc.tile_pool(name="qspool", bufs=1) as qspool,
        ):
            st = {}  # per-rep tile sets

            # ---------------- phase-1 (A) work units ----------------
            def emit_alloc(r):
                st[r] = {
                    # DoubleRow-packed q and k: one tile per m-tile (q8 for
                    # m 0-3, k8 for m 4-7), each [64, 2, N] fp8 holding the
                    # pair's heads at partition offsets 0/32, d = p + 32*plane
                    "qk8": [cpool.tile([64, 2, N], F8, name=f"qk8_{g}")
                            for g in range(MT)],
                    # per head pair [v_even | ones | v_odd]; the fused
                    # av+rowsum matmul takes a contiguous [128, 128] lhsT for
                    # either head, sharing the ones block
                    "v": [cpool.tile([128, (HPC // 2) * 3 * DH], BF16,
                                     name=f"v{j}") for j in range(NJT)],
                    "x": {},
                }

            def emit_dma_w(r):
                s = st[r]
                s["wqk"] = [wpool.tile([128, 2 * CW], BF16, name=f"wqk{k}")
                            for k in range(KT)]
                s["wv"] = [wpool.tile([128, CW], BF16, name=f"wv{k}")
                           for k in range(KT)]
                s["wo"] = [wpool.tile([128, DM], BF16, name=f"wo{c}")
                           for c in range(CW // 128)]
                for k in range(KT):
                    nc.gpsimd.dma_start(s["wqk"][k][:],
                                        wqk[k * 128:(k + 1) * 128, :])
                for k in range(KT):
                    nc.gpsimd.dma_start(s["wv"][k][:],
                                        wv[k * 128:(k + 1) * 128, :])
                for c in range(CW // 128):
                    nc.gpsimd.dma_start(s["wo"][c][:],
                                        wo[c * 128:(c + 1) * 128, :])

            def emit_dma_x(r, ch):
                x_t = xpool.tile([128, KT * NCH], BF16, name="x_t", tag="x")
                nc.sync.dma_start(x_t[:], xT[ch])
                st[r]["x"][ch] = x_t

            def emit_pq(r, ch, m):
                """The (ch, m) q|k projection chunk — 853 ns of TensorE in
                8 F=256 matmuls (smaller slices pay per-instruction
                dispatch/ldweights overhead on real hardware)."""
                s = st[r]
                x_t = s["x"][ch]
                half, lch = divmod(ch, (N // NCH) // 2)
                if lch == 0:
                    s[("qs", m, half)] = qspool.tile(
                        [128, N // 2], F8, name=f"qs{m}", tag=f"qs{m}")
                stage = s[("qs", m, half)]
                pq = psC.tile([128, NCH], F32, name="pq", tag="p1")
                for k in range(KT):
                    nc.tensor.matmul(
                        pq[:],
                        s["wqk"][k][:, m * 128:(m + 1) * 128],
                        x_t[:, k * NCH:(k + 1) * NCH],
                        start=(k == 0), stop=(k == KT - 1),
                    )
                nc.vector.tensor_copy(
                    stage[:, lch * NCH:(lch + 1) * NCH], pq[:])
                # repack half-N at a time to DoubleRow layout (SP HWDGE: the
                # per-DMA fixed cost on SWDGE/Pool would be prohibitive):
                # the m-tile's two heads land at partition offsets 0/32 of
                # qk8[m]; plane t holds d = 32*t..32*t+31
                if lch == (N // NCH) // 2 - 1:
                    hsl = slice(half * (N // 2), (half + 1) * (N // 2))
                    for hh in range(2):
                        for t in range(2):
                            nc.sync.dma_start(
                                s["qk8"][m][32 * hh:32 * (hh + 1), t, hsl],
                                stage[hh * DH + 32 * t:
                                      hh * DH + 32 * (t + 1), :],
                            )

            def emit_pv(r, ch, mt, c):
                """Two head-pair column blocks (256 of 512 v-cols) of the
                (ch, mt) v projection — 853 ns of TensorE."""
                s = st[r]
                x_t = s["x"][ch]
                j = ch * (NCH // 128) + mt
                pv = psC.tile([128, 256], F32, name="pv", tag="p1")
                for k in range(KT):
                    nc.tensor.matmul(
                        pv[:],
                        x_t[:, k * NCH + mt * 128:k * NCH + (mt + 1) * 128],
                        s["wv"][k][:, c * 256:(c + 1) * 256],
                        start=(k == 0), stop=(k == KT - 1),
                    )
                v3 = s["v"][j].rearrange("p (q c) -> p q c", c=3 * DH)
                pv3 = pv[:].rearrange("p (l c) -> p l c", c=DH)
                nc.vector.tensor_copy(v3[:, 2 * c:2 * c + 2, 0:DH],
                                      pv3[:, 0::2, :])
                nc.vector.tensor_copy(v3[:, 2 * c:2 * c + 2, 2 * DH:3 * DH],
                                      pv3[:, 1::2, :])
                if c == 0:
                    nc.any.memset(v3[:, :, DH:2 * DH], 1.0)

            def a_units(r):
                units = deque()
                units.append((0.0, lambda: emit_alloc(r)))
                units.append((0.0, lambda: emit_dma_x(r, 0)))
                units.append((0.0, lambda: emit_dma_w(r)))
                units.append((0.0, lambda: emit_dma_x(r, 1)))
                for ch in range(N // NCH):
                    if ch + 2 < N // NCH:
                        units.append(
                            (0.0, lambda ch=ch: emit_dma_x(r, ch + 2)))
                    for m in range(MT):
                        units.append(
                            (853.0, lambda ch=ch, m=m: emit_pq(r, ch, m)))
                    for mt in range(NCH // 128):
                        for c in range(2):
                            units.append(
                                (853.0, lambda ch=ch, mt=mt, c=c:
                                 emit_pv(r, ch, mt, c)))
                return units

            # ---------------- phase-2/3 (B) emission ----------------
            def emit_scores(r, p, ic, jt):
                s = st[r]
                isl = slice(ic * ICH, (ic + 1) * ICH)
                s_ps = psA.tile([128, 2 * ICH], F32, name="s_ps", tag="s")
                for half in range(2):
                    po = 32 * half
                    q8 = s["qk8"][p]
                    k8 = s["qk8"][MT // 2 + p]
                    nc.tensor.matmul(
                        s_ps[:, half * ICH:(half + 1) * ICH],
                        k8[po:po + 32, :, jt * 128:(jt + 1) * 128],
                        q8[po:po + 32, :, isl],
                        start=True, stop=True,
                        perf_mode=PM,
                    )
                ex = epool.tile([128, 2 * ICH], BF16, name="ex", tag="ex")
                nc.scalar.activation(ex[:], s_ps[:], AF.Exp, scale=0.125)
                return ex

            def emit_av(r, p, jt, ex, av2):
                s = st[r]
                for half in range(2):
                    base = p * 3 * DH + half * DH
                    vl = s["v"][jt][:, base:base + 2 * DH]
                    nc.tensor.matmul(
                        av2[:, half * ICH:(half + 1) * ICH],
                        vl,
                        ex[:, half * ICH:(half + 1) * ICH],
                        start=(jt == 0), stop=(jt == NJT - 1),
                    )

            def emit_normalize(r, p, ic, av2):
                s = st[r]
                isl = slice(ic * ICH, (ic + 1) * ICH)
                # one fast copy releases the PSUM accumulator for the next
                # stream; the reciprocal/mul chain then runs off-critical-path
                # from the SBUF scratch
                avs = lpool.tile([128, 2 * ICH], F32, name="avs", tag="avs",
                                 bufs=2)
                nc.vector.tensor_copy(avs[:], av2[:])
                for half in range(2):
                    l = 2 * p + half
                    ct, coff = l // 2, (l % 2) * DH
                    # even head: rows [out | sums]; odd head: [sums | out]
                    o0, s0 = (0, DH) if half == 0 else (DH, 0)
                    hsl = slice(half * ICH, (half + 1) * ICH)
                    # rc rows sit at the out-rows' base partition: the DVE
                    # mul requires equal base partitions for two SBUF inputs
                    rc = lpool.tile([128, ICH], F32, name="rc", tag="rc",
                                    bufs=2)
                    nc.vector.reciprocal(rc[o0:o0 + DH, :],
                                         avs[s0:s0 + DH, hsl])
                    nc.vector.tensor_mul(
                        s["aoT"][ct][coff:coff + DH, isl],
                        avs[o0:o0 + DH, hsl],
                        rc[o0:o0 + DH, :],
                    )

            def emit_ph3(r, nt, h):
                s = st[r]
                po = psC.tile([128, 512], F32, name="po", tag="p1")
                for c in range(CW // 128):
                    nc.tensor.matmul(
                        po[:],
                        s["aoT"][c][:, nt * 128:(nt + 1) * 128],
                        s["wo"][c][:, h * 512:(h + 1) * 512],
                        start=(c == 0), stop=(c == CW // 128 - 1),
                    )
                os_ = lpool.tile([128, 512], F32, name="os", tag="os")
                nc.vector.tensor_copy(os_[:], po[:])
                nc.scalar.dma_start(
                    out[nt * 128:(nt + 1) * 128, h * 512:(h + 1) * 512],
                    os_[:],
                )

            def emit_b(r, fillers):
                """Cell stream for rep r, weaving filler units into TensorE
                slack. fillers: deque of (cost_ns, emit_fn) — leftovers from
                the previous rep followed by the next rep's phase-1 units.
                This rep's ph3 groups are generated internally and take
                priority for credit pops; forced pops (stream-boundary PE
                spacers) pull phase-1 units so they never stall on the
                normalize just issued. Returns the unconsumed queue."""
                st[r]["aoT"] = [lpool.tile([128, N], BF16, name=f"aoT{c}",
                                           bufs=1) for c in range(CW // 128)]
                ph3q = deque()
                credit = 0.0
                cells = [(p, ic, jt) for ic in range(NIC)
                         for p in range(HPC // 2) for jt in range(NJT)]
                av_cur = None
                pend = None  # (p, ic, jt, ex, av2)

                def pop_filler(force=False):
                    nonlocal credit
                    if force:
                        q = fillers if fillers else ph3q
                    else:
                        q = ph3q if ph3q else fillers
                    if not q:
                        return False
                    cost, fn = q[0]
                    if not force and credit < cost:
                        return False
                    q.popleft()
                    fn()
                    if not force:
                        credit -= cost
                    return True

                def finish(cell):
                    nonlocal credit
                    p, ic, jt, ex, av2 = cell
                    emit_av(r, p, jt, ex, av2)
                    credit = min(credit + SLACK, CREDIT_CAP)
                    if jt == NJT - 1:
                        emit_normalize(r, p, ic, av2)
                        if p == HPC // 2 - 1:
                            for nt in range(ic * ICH // 128,
                                            (ic + 1) * ICH // 128):
                                for h in range(DM // 512):
                                    ph3q.append((853.0,
                                                 lambda nt=nt, h=h:
                                                 emit_ph3(r, nt, h)))
                        # PE spacer while DVE copies the accumulator out, so
                        # the next stream's first av-accumulate doesn't
                        # head-of-line stall on the "av" WAR
                        pop_filler(force=True)
                    else:
                        while pop_filler():
                            pass

                for (p, ic, jt) in cells:
                    if jt == 0:
                        av_cur = psB.tile([128, 2 * ICH], F32, name="av",
                                          tag="av")
                    ex = emit_scores(r, p, ic, jt)
                    if pend is not None:
                        finish(pend)
                    pend = (p, ic, jt, ex, av_cur)
                finish(pend)
                leftover = deque(ph3q)
                leftover.extend(fillers)
                return leftover

            # ---------------- weave reps ----------------
            boot = a_units(0)
            while boot:
                boot.popleft()[1]()  # A(0) solo
            carry = deque()
            for r in range(reps):
                fillers = deque(carry)
                if r + 1 < reps:
                    fillers.extend(a_units(r + 1))
                # rep r's tile set must exist before its cell stream starts
                while r not in st and fillers:
                    fillers.popleft()[1]()
                carry = emit_b(r, fillers)
            while carry:
                carry.popleft()[1]()

    nc.finalize()
    return nc


def make_in_maps(inputs_np):
    bf16 = mybir.dt.np(BF16)
    x = np.ascontiguousarray(inputs_np["x"], dtype=np.float32)
    w_qkv = np.asarray(inputs_np["w_qkv"], dtype=np.float32)
    w_out = np.asarray(inputs_np["w_out"], dtype=np.float32)
    in_maps = []
    xp_cache = {}
    for core in range(8):
        b, g = divmod(core, 2)
        if b not in xp_cache:
            # pack x[b]^T as [chunk, partition, k, n] so device chunk loads
            # are single linear DMAs
            xp_cache[b] = np.ascontiguousarray(
                x[b].T.reshape(KT, 128, N // NCH, NCH).transpose(2, 1, 0, 3)
            ).reshape(N // NCH, 128, KT * NCH).astype(bf16)
        xTb = xp_cache[b]
        wq = w_qkv[:, g * CW:(g + 1) * CW]
        wk = w_qkv[:, DM + g * CW:DM + (g + 1) * CW]
        wv_ = w_qkv[:, 2 * DM + g * CW:2 * DM + (g + 1) * CW]
        in_maps.append({
            "xT": xTb,
            "wqk": np.ascontiguousarray(
                np.concatenate([wq, wk], axis=1)).astype(bf16),
            "wv": np.ascontiguousarray(wv_).astype(bf16),
            "wo": np.ascontiguousarray(
                w_out[g * CW:(g + 1) * CW, :]).astype(bf16),
        })
    return in_maps


_NC_CACHE = {}


def _get_nc():
    if "nc" not in _NC_CACHE:
        _NC_CACHE["nc"] = build_nc()
    return _NC_CACHE["nc"]


def kernel(x, w_qkv, w_out, b_out):
    b_out = np.asarray(b_out, dtype=np.float32)
    nc = _get_nc()
    in_maps = make_in_maps({"x": x, "w_qkv": w_qkv, "w_out": w_out})
    res = run_bass_kernel_spmd(nc, in_maps, core_ids=list(range(8)))
    _NC_CACHE["last_result"] = res
    out = np.empty((B, N, DM), np.float32)
    for b in range(B):
        out[b] = res.results[2 * b]["out"] + res.results[2 * b + 1]["out"] + b_out
    return out


# revision 7
# speedup vs baseline: 1.1285x; 1.1285x over previous
"""Multi-head attention on 8 Trainium2 NeuronCores — v3 (woven pipeline).

Problem: x[4, 2048, 1024], 16 heads x 64 dim.
  qkv = x @ w_qkv; attn = softmax(q k^T / 8); out = (attn v) @ w_out + b_out

Sharding: 8 cores = 4 batches x 2 head-groups (8 heads each); host sums
the two partial out-projections per batch and adds the bias.

Engine model (from the cost model + TimelineSim): per attention cell
(pair, i-chunk, j-tile) the exp on ScalarE costs ~1038 ns vs ~853 ns of
TensorE work, so the 256-cell phase 2 is Activation-bound while TensorE
idles ~185 ns/cell; phases 1/3 are TensorE-only while ScalarE idles.
Engines execute their queues in order, so overlap must be programmed in
EMIT order. v3 therefore weaves one global instruction stream:

  - cells emit as [scores(n+1) | av(n)] (1-cell software pipeline) so the
    exp chain on ScalarE never waits on TensorE;
  - a slack-credit weaver inserts "filler" TensorE work between cells:
    this rep's out-projection groups and THE NEXT REP's phase-1 chunks
    (cross-rep pipelining; cpool bufs=2 double-buffers qkT/v across reps);
  - all inputs/SBUF tensors bf16 (PSUM fp32), halving DMA and SBUF;
  - one [128, 2*ICH] PSUM accumulator rotates across the 16 (pair, ic)
    streams; two forced fillers after each normalize cover its WAR gap.
"""

from collections import deque

import numpy as np

import concourse.bacc as bacc
import concourse.mybir as mybir
import concourse.tile as tile
from concourse.bass_utils import run_bass_kernel_spmd

F32 = mybir.dt.float32
BF16 = mybir.dt.bfloat16
F8 = mybir.dt.float8e4
PM = mybir.MatmulPerfMode.DoubleRow
AF = mybir.ActivationFunctionType

B = 4          # batch
N = 2048       # sequence
DM = 1024      # model dim
NH = 16        # heads
DH = 64        # head dim
G = 2          # head groups (cores per batch)
HPC = NH // G  # heads per core = 8
CW = DH * HPC  # per-core qkv column width = 512

NCH = 256      # phase-1 x^T column chunk
ICH = 512      # phase-2 i (query) chunk (per head; a pair shares [128, 2*ICH])

KT = DM // 128      # 8 contraction tiles over d
MT = 2 * CW // 128  # 8 c-tiles for q|k
NJT = N // 128      # 16 j tiles
NIC = N // ICH      # 4 i chunks

SLACK = 450.0       # ns of TensorE slack banked per cell for filler work
CREDIT_CAP = 1400.0  # caps filler bursts that would starve the exp chain


def build_nc(reps=1):
    nc = bacc.Bacc(None, target_bir_lowering=False, debug=False)

    xT = nc.declare_dram_parameter("xT", [N // NCH, 128, KT * NCH], BF16,
                                   isOutput=False)
    wqk = nc.declare_dram_parameter("wqk", [DM, 2 * CW], BF16, isOutput=False)
    wv = nc.declare_dram_parameter("wv", [DM, CW], BF16, isOutput=False)
    wo = nc.declare_dram_parameter("wo", [CW, DM], BF16, isOutput=False)
    out = nc.declare_dram_parameter("out", [N, DM], F32, isOutput=True)

    with tile.TileContext(nc) as tc:
        with (
            # cross-rep double buffering for the phase-1 products
            tc.tile_pool(name="cpool", bufs=2) as cpool,
            # 8 PSUM banks: "s" 2x[128,1024] scores, "av" 1x[128,1024]
            # attention accumulator, "p1" 2x[128,512] projections
            tc.tile_pool(name="psA", bufs=2, space="PSUM") as psA,
            tc.tile_pool(name="psB", bufs=1, space="PSUM") as psB,
            tc.tile_pool(name="psC", bufs=2, space="PSUM") as psC,
            tc.tile_pool(name="epool", bufs=3) as epool,
            tc.tile_pool(name="wpool", bufs=1) as wpool,
            tc.tile_pool(name="lpool", bufs=2) as lpool,
            tc.tile_pool(name="xpool", bufs=2) as xpool,
        ):
            st = {}  # per-rep tile sets

            # ---------------- phase-1 (A) work units ----------------
            def emit_alloc(r):
                st[r] = {
                    # q for pairs 0-3, two heads stacked on partitions
                    "qT": [cpool.tile([128, N], BF16, name=f"qT{m}")
                           for m in range(MT // 2)],
                    # zero-padded k: plane 0 holds [k_even; 0], plane 1
                    # [0; k_odd], so score matmuls contract the full 128
                    # partitions (K=64 matmuls run at half rate on HW) with
                    # the full [q_e; q_o] slice as the moving operand
                    "kpad": [cpool.tile([128, 2, N], BF16, name=f"kpad{g}")
                             for g in range(MT // 2)],
                    # per head pair [v_even | ones | v_odd]; the fused
                    # av+rowsum matmul takes a contiguous [128, 128] lhsT for
                    # either head, sharing the ones block
                    "v": [cpool.tile([128, (HPC // 2) * 3 * DH], BF16,
                                     name=f"v{j}") for j in range(NJT)],
                    "x": {},
                }
                if r < 2:
                    # the pad rows are zero in both cpool buffers forever
                    # after the first two reps write them
                    for g in range(MT // 2):
                        kp = st[r]["kpad"][g]
                        nc.any.memset(kp[DH:128, 0, :], 0.0)
                        nc.any.memset(kp[0:DH, 1, :], 0.0)

            def emit_dma_w(r):
                s = st[r]
                s["wqk"] = [wpool.tile([128, 2 * CW], BF16, name=f"wqk{k}")
                            for k in range(KT)]
                s["wv"] = [wpool.tile([128, CW], BF16, name=f"wv{k}")
                           for k in range(KT)]

                for k in range(KT):
                    nc.gpsimd.dma_start(s["wqk"][k][:],
                                        wqk[k * 128:(k + 1) * 128, :])
                for k in range(KT):
                    nc.gpsimd.dma_start(s["wv"][k][:],
                                        wv[k * 128:(k + 1) * 128, :])


            def emit_dma_wo(r):
                s = st[r]
                s["wo"] = [wpool.tile([128, DM], BF16, name=f"wo{c}")
                           for c in range(CW // 128)]
                for c in range(CW // 128):
                    nc.gpsimd.dma_start(s["wo"][c][:],
                                        wo[c * 128:(c + 1) * 128, :])

            def emit_dma_x(r, ch):
                x_t = xpool.tile([128, KT * NCH], BF16, name="x_t", tag="x")
                nc.sync.dma_start(x_t[:], xT[ch])
                st[r]["x"][ch] = x_t

            def emit_pq(r, ch, m):
                """The (ch, m) q|k projection chunk — 853 ns of TensorE in
                8 F=256 matmuls (smaller slices pay per-instruction
                dispatch/ldweights overhead on real hardware)."""
                s = st[r]
                x_t = s["x"][ch]
                pq = psC.tile([128, NCH], F32, name="pq", tag="p1")
                for k in range(KT):
                    nc.tensor.matmul(
                        pq[:],
                        s["wqk"][k][:, m * 128:(m + 1) * 128],
                        x_t[:, k * NCH:(k + 1) * NCH],
                        start=(k == 0), stop=(k == KT - 1),
                    )
                csl = slice(ch * NCH, (ch + 1) * NCH)
                if m < MT // 2:
                    nc.vector.tensor_copy(s["qT"][m][:, csl], pq[:])
                else:
                    kp = s["kpad"][m - MT // 2]
                    nc.vector.tensor_copy(kp[0:DH, 0, csl], pq[0:DH, :])
                    nc.vector.tensor_copy(kp[DH:128, 1, csl], pq[DH:128, :])

            def emit_pv(r, ch, mt, c):
                """Two head-pair column blocks (256 of 512 v-cols) of the
                (ch, mt) v projection — 853 ns of TensorE."""
                s = st[r]
                x_t = s["x"][ch]
                j = ch * (NCH // 128) + mt
                pv = psC.tile([128, 256], F32, name="pv", tag="p1")
                for k in range(KT):
                    nc.tensor.matmul(
                        pv[:],
                        x_t[:, k * NCH + mt * 128:k * NCH + (mt + 1) * 128],
                        s["wv"][k][:, c * 256:(c + 1) * 256],
                        start=(k == 0), stop=(k == KT - 1),
                    )
                v3 = s["v"][j].rearrange("p (q c) -> p q c", c=3 * DH)
                pv3 = pv[:].rearrange("p (l c) -> p l c", c=DH)
                nc.vector.tensor_copy(v3[:, 2 * c:2 * c + 2, 0:DH],
                                      pv3[:, 0::2, :])
                nc.vector.tensor_copy(v3[:, 2 * c:2 * c + 2, 2 * DH:3 * DH],
                                      pv3[:, 1::2, :])
                if c == 0:
                    nc.any.memset(v3[:, :, DH:2 * DH], 1.0)

            def a_units(r):
                units = deque()
                units.append((0.0, lambda: emit_alloc(r)))
                units.append((0.0, lambda: emit_dma_x(r, 0)))
                units.append((0.0, lambda: emit_dma_w(r)))
                units.append((0.0, lambda: emit_dma_x(r, 1)))
                for ch in range(N // NCH):
                    if ch + 2 < N // NCH:
                        units.append(
                            (0.0, lambda ch=ch: emit_dma_x(r, ch + 2)))
                    for m in range(MT):
                        units.append(
                            (460.0, lambda ch=ch, m=m: emit_pq(r, ch, m)))
                    for mt in range(NCH // 128):
                        for c in range(2):
                            units.append(
                                (460.0, lambda ch=ch, mt=mt, c=c:
                                 emit_pv(r, ch, mt, c)))
                units.append((0.0, lambda: emit_dma_wo(r)))
                return units

            # ---------------- phase-2/3 (B) emission ----------------
            def emit_scores(r, p, ic, jt):
                s = st[r]
                isl = slice(ic * ICH, (ic + 1) * ICH)
                s_ps = psA.tile([128, 2 * ICH], F32, name="s_ps", tag="s")
                qt, kp = s["qT"][p], s["kpad"][p]
                for half in range(2):
                    nc.tensor.matmul(
                        s_ps[:, half * ICH:(half + 1) * ICH],
                        kp[:, half, jt * 128:(jt + 1) * 128],
                        qt[:, isl],
                        start=True, stop=True,
                    )
                ex = epool.tile([128, 2 * ICH], BF16, name="ex", tag="ex")
                nc.scalar.activation(ex[:], s_ps[:], AF.Exp, scale=0.125)
                return ex

            def emit_av(r, p, jt, ex, av2):
                s = st[r]
                for half in range(2):
                    base = p * 3 * DH + half * DH
                    vl = s["v"][jt][:, base:base + 2 * DH]
                    nc.tensor.matmul(
                        av2[:, half * ICH:(half + 1) * ICH],
                        vl,
                        ex[:, half * ICH:(half + 1) * ICH],
                        start=(jt == 0), stop=(jt == NJT - 1),
                    )

            def emit_normalize(r, p, ic, av2):
                s = st[r]
                if p == 0:
                    # per-ic out rows; bufs=2 so the next ic's normalize can
                    # start while this ic's out-projection drains
                    s["aoT"] = [lpool.tile([128, ICH], BF16, name=f"aoT{c}",
                                           tag=f"aoT{c}", bufs=2)
                                for c in range(CW // 128)]
                # one fast copy releases the PSUM accumulator for the next
                # stream; the reciprocal/mul chain then runs off-critical-path
                # from the SBUF scratch
                avs = lpool.tile([128, 2 * ICH], BF16, name="avs",
                                 tag="avs", bufs=2)
                with nc.allow_low_precision(
                        reason="bf16 softmax num/den; error ~0.1% rms, "
                               "well under the 2e-2 gate"):
                    nc.vector.tensor_copy(avs[:], av2[:])
                for half in range(2):
                    l = 2 * p + half
                    ct, coff = l // 2, (l % 2) * DH
                    # even head: rows [out | sums]; odd head: [sums | out]
                    o0, s0 = (0, DH) if half == 0 else (DH, 0)
                    hsl = slice(half * ICH, (half + 1) * ICH)
                    # rc rows sit at the out-rows' base partition: the DVE
                    # mul requires equal base partitions for two SBUF inputs
                    rc = lpool.tile([128, ICH], BF16, name="rc", tag="rc",
                                    bufs=1)
                    with nc.allow_low_precision(
                            reason="bf16 softmax normalize; ~0.1% rms"):
                        nc.vector.reciprocal(rc[o0:o0 + DH, :],
                                             avs[s0:s0 + DH, hsl])
                        nc.vector.tensor_mul(
                            s["aoT"][ct][coff:coff + DH, :],
                            avs[o0:o0 + DH, hsl],
                            rc[o0:o0 + DH, :],
                        )

            def emit_ph3(r, nt, h, ao):
                s = st[r]
                po = psC.tile([128, 512], F32, name="po", tag="p1")
                lnt = nt % (ICH // 128)
                for c in range(CW // 128):
                    nc.tensor.matmul(
                        po[:],
                        ao[c][:, lnt * 128:(lnt + 1) * 128],
                        s["wo"][c][:, h * 512:(h + 1) * 512],
                        start=(c == 0), stop=(c == CW // 128 - 1),
                    )
                os_ = lpool.tile([128, 512], F32, name="os", tag="os")
                nc.vector.tensor_copy(os_[:], po[:])
                nc.scalar.dma_start(
                    out[nt * 128:(nt + 1) * 128, h * 512:(h + 1) * 512],
                    os_[:],
                )

            def emit_b(r, fillers):
                """Cell stream for rep r, weaving filler units into TensorE
                slack. fillers: deque of (cost_ns, emit_fn) — leftovers from
                the previous rep followed by the next rep's phase-1 units.
                This rep's ph3 groups are generated internally and take
                priority for credit pops; forced pops (stream-boundary PE
                spacers) pull phase-1 units so they never stall on the
                normalize just issued. Returns the unconsumed queue."""

                ph3q = deque()
                credit = 0.0
                cells = [(p, ic, jt) for ic in range(NIC)
                         for p in range(HPC // 2) for jt in range(NJT)]
                av_cur = None
                pend = None  # (p, ic, jt, ex, av2)

                def pop_filler(force=False):
                    nonlocal credit
                    if force:
                        q = fillers if fillers else ph3q
                    else:
                        q = ph3q if ph3q else fillers
                    if not q:
                        return False
                    cost, fn = q[0]
                    if not force and credit < cost:
                        return False
                    q.popleft()
                    fn()
                    if not force:
                        credit -= cost
                    return True

                def finish(cell):
                    nonlocal credit
                    p, ic, jt, ex, av2 = cell
                    emit_av(r, p, jt, ex, av2)
                    credit = min(credit + SLACK, CREDIT_CAP)
                    if jt == NJT - 1:
                        emit_normalize(r, p, ic, av2)
                        if p == HPC // 2 - 1:
                            for nt in range(ic * ICH // 128,
                                            (ic + 1) * ICH // 128):
                                for h in range(DM // 512):
                                    ph3q.append((600.0,
                                                 lambda nt=nt, h=h,
                                                 ao=st[r]["aoT"]:
                                                 emit_ph3(r, nt, h, ao)))
                        # PE spacer while DVE copies the accumulator out, so
                        # the next stream's first av-accumulate doesn't
                        # head-of-line stall on the "av" WAR
                        pop_filler(force=True)
                    else:
                        while pop_filler():
                            pass

                for (p, ic, jt) in cells:
                    if jt == 0:
                        av_cur = psB.tile([128, 2 * ICH], F32, name="av",
                                          tag="av")
                    ex = emit_scores(r, p, ic, jt)
                    if pend is not None:
                        finish(pend)
                    pend = (p, ic, jt, ex, av_cur)
                finish(pend)
                leftover = deque(ph3q)
                leftover.extend(fillers)
                return leftover

            # ---------------- weave reps ----------------
            boot = a_units(0)
            while boot:
                boot.popleft()[1]()  # A(0) solo
            carry = deque()
            for r in range(reps):
                fillers = deque(carry)
                if r + 1 < reps:
                    fillers.extend(a_units(r + 1))
                # rep r's tile set must exist before its cell stream starts
                while r not in st and fillers:
                    fillers.popleft()[1]()
                carry = emit_b(r, fillers)
            while carry:
                carry.popleft()[1]()

    nc.finalize()
    return nc


def make_in_maps(inputs_np):
    bf16 = mybir.dt.np(BF16)
    x = np.ascontiguousarray(inputs_np["x"], dtype=np.float32)
    w_qkv = np.asarray(inputs_np["w_qkv"], dtype=np.float32)
    w_out = np.asarray(inputs_np["w_out"], dtype=np.float32)
    in_maps = []
    xp_cache = {}
    for core in range(8):
        b, g = divmod(core, 2)
        if b not in xp_cache:
            # pack x[b]^T as [chunk, partition, k, n] so device chunk loads
            # are single linear DMAs
            xp_cache[b] = np.ascontiguousarray(
                x[b].T.reshape(KT, 128, N // NCH, NCH).transpose(2, 1, 0, 3)
            ).reshape(N // NCH, 128, KT * NCH).astype(bf16)
        xTb = xp_cache[b]
        wq = w_qkv[:, g * CW:(g + 1) * CW]
        wk = w_qkv[:, DM + g * CW:DM + (g + 1) * CW]
        wv_ = w_qkv[:, 2 * DM + g * CW:2 * DM + (g + 1) * CW]
        in_maps.append({
            "xT": xTb,
            "wqk": np.ascontiguousarray(
                np.concatenate([wq, wk], axis=1)).astype(bf16),
            "wv": np.ascontiguousarray(wv_).astype(bf16),
            "wo": np.ascontiguousarray(
                w_out[g * CW:(g + 1) * CW, :]).astype(bf16),
        })
    return in_maps


_NC_CACHE = {}


def _get_nc():
    if "nc" not in _NC_CACHE:
        _NC_CACHE["nc"] = build_nc()
    return _NC_CACHE["nc"]


def kernel(x, w_qkv, w_out, b_out):
    b_out = np.asarray(b_out, dtype=np.float32)
    nc = _get_nc()
    in_maps = make_in_maps({"x": x, "w_qkv": w_qkv, "w_out": w_out})
    res = run_bass_kernel_spmd(nc, in_maps, core_ids=list(range(8)))
    _NC_CACHE["last_result"] = res
    out = np.empty((B, N, DM), np.float32)
    for b in range(B):
        out[b] = res.results[2 * b]["out"] + res.results[2 * b + 1]["out"] + b_out
    return out


# revision 8
# speedup vs baseline: 1.3148x; 1.1650x over previous
"""Multi-head attention on 8 Trainium2 NeuronCores — v3 (woven pipeline).

Problem: x[4, 2048, 1024], 16 heads x 64 dim.
  qkv = x @ w_qkv; attn = softmax(q k^T / 8); out = (attn v) @ w_out + b_out

Sharding: 8 cores = 4 batches x 2 head-groups (8 heads each); host sums
the two partial out-projections per batch and adds the bias.

Engine model (from the cost model + TimelineSim): per attention cell
(pair, i-chunk, j-tile) the exp on ScalarE costs ~1038 ns vs ~853 ns of
TensorE work, so the 256-cell phase 2 is Activation-bound while TensorE
idles ~185 ns/cell; phases 1/3 are TensorE-only while ScalarE idles.
Engines execute their queues in order, so overlap must be programmed in
EMIT order. v3 therefore weaves one global instruction stream:

  - cells emit as [scores(n+1) | av(n)] (1-cell software pipeline) so the
    exp chain on ScalarE never waits on TensorE;
  - a slack-credit weaver inserts "filler" TensorE work between cells:
    this rep's out-projection groups and THE NEXT REP's phase-1 chunks
    (cross-rep pipelining; cpool bufs=2 double-buffers qkT/v across reps);
  - all inputs/SBUF tensors bf16 (PSUM fp32), halving DMA and SBUF;
  - one [128, 2*ICH] PSUM accumulator rotates across the 16 (pair, ic)
    streams; two forced fillers after each normalize cover its WAR gap.
"""

from collections import deque

import numpy as np

import concourse.bacc as bacc
import concourse.mybir as mybir
import concourse.tile as tile
from concourse.bass_utils import run_bass_kernel_spmd

F32 = mybir.dt.float32
BF16 = mybir.dt.bfloat16
F8 = mybir.dt.float8e4
PM = mybir.MatmulPerfMode.DoubleRow
AF = mybir.ActivationFunctionType

B = 4          # batch
N = 2048       # sequence
DM = 1024      # model dim
NH = 16        # heads
DH = 64        # head dim
G = 2          # head groups (cores per batch)
HPC = NH // G  # heads per core = 8
CW = DH * HPC  # per-core qkv column width = 512

NCH = 256      # phase-1 x^T column chunk
ICH = 512      # phase-2 i (query) chunk (per head; a pair shares [128, 2*ICH])

KT = DM // 128      # 8 contraction tiles over d
MT = 2 * CW // 128  # 8 c-tiles for q|k
NJT = N // 128      # 16 j tiles
NIC = N // ICH      # 4 i chunks

SLACK = 450.0       # ns of TensorE slack banked per cell for filler work
CREDIT_CAP = 1400.0  # caps filler bursts that would starve the exp chain


def build_nc(reps=1):
    nc = bacc.Bacc(None, target_bir_lowering=False, debug=False)

    xT = nc.declare_dram_parameter("xT", [N // NCH, 128, KT * NCH], BF16,
                                   isOutput=False)
    wqk = nc.declare_dram_parameter("wqk", [DM, 2 * CW], BF16, isOutput=False)
    wv = nc.declare_dram_parameter("wv", [DM, CW], BF16, isOutput=False)
    wo = nc.declare_dram_parameter("wo", [CW, DM], BF16, isOutput=False)
    out = nc.declare_dram_parameter("out", [N, DM], F32, isOutput=True)

    with tile.TileContext(nc) as tc:
        with (
            # cross-rep double buffering for the phase-1 products
            tc.tile_pool(name="cpool", bufs=2) as cpool,
            # 8 PSUM banks: "s" 2x[128,1024] scores, "av" 1x[128,1024]
            # attention accumulator, "p1" 2x[128,512] projections
            tc.tile_pool(name="psA", bufs=2, space="PSUM") as psA,
            tc.tile_pool(name="psB", bufs=1, space="PSUM") as psB,
            tc.tile_pool(name="psC", bufs=2, space="PSUM") as psC,
            tc.tile_pool(name="epool", bufs=5) as epool,
            tc.tile_pool(name="wpool", bufs=1) as wpool,
            tc.tile_pool(name="lpool", bufs=2) as lpool,
            tc.tile_pool(name="xpool", bufs=2) as xpool,
        ):
            st = {}  # per-rep tile sets

            # ---------------- phase-1 (A) work units ----------------
            def emit_alloc(r):
                st[r] = {
                    # q for pairs 0-3, two heads stacked on partitions
                    "qT": [cpool.tile([128, N], BF16, name=f"qT{m}")
                           for m in range(MT // 2)],
                    # zero-padded k: plane 0 holds [k_even; 0], plane 1
                    # [0; k_odd], so score matmuls contract the full 128
                    # partitions (K=64 matmuls run at half rate on HW) with
                    # the full [q_e; q_o] slice as the moving operand
                    "kpad": [cpool.tile([128, 2, N], BF16, name=f"kpad{g}")
                             for g in range(MT // 2)],
                    # per head pair [v_even | ones | v_odd]; the fused
                    # av+rowsum matmul takes a contiguous [128, 128] lhsT for
                    # either head, sharing the ones block
                    "v": [cpool.tile([128, (HPC // 2) * 3 * DH], BF16,
                                     name=f"v{j}") for j in range(NJT)],
                    "x": {},
                }
                if r < 2:
                    # the pad rows are zero in both cpool buffers forever
                    # after the first two reps write them
                    for g in range(MT // 2):
                        kp = st[r]["kpad"][g]
                        nc.any.memset(kp[DH:128, 0, :], 0.0)
                        nc.any.memset(kp[0:DH, 1, :], 0.0)

            def emit_dma_w(r):
                s = st[r]
                s["wqk"] = [wpool.tile([128, 2 * CW], BF16, name=f"wqk{k}")
                            for k in range(KT)]
                s["wv"] = [wpool.tile([128, CW], BF16, name=f"wv{k}")
                           for k in range(KT)]

                for k in range(KT):
                    nc.gpsimd.dma_start(s["wqk"][k][:],
                                        wqk[k * 128:(k + 1) * 128, :])
                for k in range(KT):
                    nc.gpsimd.dma_start(s["wv"][k][:],
                                        wv[k * 128:(k + 1) * 128, :])


            def emit_dma_wo(r):
                s = st[r]
                s["wo"] = [wpool.tile([128, DM], BF16, name=f"wo{c}")
                           for c in range(CW // 128)]
                for c in range(CW // 128):
                    nc.gpsimd.dma_start(s["wo"][c][:],
                                        wo[c * 128:(c + 1) * 128, :])

            def emit_dma_x(r, ch):
                x_t = xpool.tile([128, KT * NCH], BF16, name="x_t", tag="x")
                nc.sync.dma_start(x_t[:], xT[ch])
                st[r]["x"][ch] = x_t

            def emit_pq(r, ch, m):
                """The (ch, m) q|k projection chunk — 853 ns of TensorE in
                8 F=256 matmuls (smaller slices pay per-instruction
                dispatch/ldweights overhead on real hardware)."""
                s = st[r]
                x_t = s["x"][ch]
                pq = psC.tile([128, NCH], F32, name="pq", tag="p1")
                for k in range(KT):
                    nc.tensor.matmul(
                        pq[:],
                        s["wqk"][k][:, m * 128:(m + 1) * 128],
                        x_t[:, k * NCH:(k + 1) * NCH],
                        start=(k == 0), stop=(k == KT - 1),
                    )
                csl = slice(ch * NCH, (ch + 1) * NCH)
                if m < MT // 2:
                    nc.vector.tensor_copy(s["qT"][m][:, csl], pq[:])
                else:
                    kp = s["kpad"][m - MT // 2]
                    nc.vector.tensor_copy(kp[0:DH, 0, csl], pq[0:DH, :])
                    nc.vector.tensor_copy(kp[DH:128, 1, csl], pq[DH:128, :])

            def emit_pv(r, ch, mt, c):
                """Two head-pair column blocks (256 of 512 v-cols) of the
                (ch, mt) v projection — 853 ns of TensorE."""
                s = st[r]
                x_t = s["x"][ch]
                j = ch * (NCH // 128) + mt
                pv = psC.tile([128, 256], F32, name="pv", tag="p1")
                for k in range(KT):
                    nc.tensor.matmul(
                        pv[:],
                        x_t[:, k * NCH + mt * 128:k * NCH + (mt + 1) * 128],
                        s["wv"][k][:, c * 256:(c + 1) * 256],
                        start=(k == 0), stop=(k == KT - 1),
                    )
                v3 = s["v"][j].rearrange("p (q c) -> p q c", c=3 * DH)
                pv3 = pv[:].rearrange("p (l c) -> p l c", c=DH)
                nc.vector.tensor_copy(v3[:, 2 * c:2 * c + 2, 0:DH],
                                      pv3[:, 0::2, :])
                nc.vector.tensor_copy(v3[:, 2 * c:2 * c + 2, 2 * DH:3 * DH],
                                      pv3[:, 1::2, :])
                if c == 0:
                    nc.any.memset(v3[:, :, DH:2 * DH], 1.0)

            def a_units(r):
                units = deque()
                units.append((0.0, lambda: emit_alloc(r)))
                units.append((0.0, lambda: emit_dma_x(r, 0)))
                units.append((0.0, lambda: emit_dma_w(r)))
                units.append((0.0, lambda: emit_dma_x(r, 1)))
                for ch in range(N // NCH):
                    if ch + 2 < N // NCH:
                        units.append(
                            (0.0, lambda ch=ch: emit_dma_x(r, ch + 2)))
                    for m in range(MT):
                        units.append(
                            (460.0, lambda ch=ch, m=m: emit_pq(r, ch, m)))
                    for mt in range(NCH // 128):
                        for c in range(2):
                            units.append(
                                (460.0, lambda ch=ch, mt=mt, c=c:
                                 emit_pv(r, ch, mt, c)))
                units.append((0.0, lambda: emit_dma_wo(r)))
                return units

            # ---------------- phase-2/3 (B) emission ----------------
            def emit_scores(r, p, ic, jt):
                s = st[r]
                isl = slice(ic * ICH, (ic + 1) * ICH)
                s_ps = psA.tile([128, 2 * ICH], F32, name="s_ps", tag="s")
                qt, kp = s["qT"][p], s["kpad"][p]
                for half in range(2):
                    nc.tensor.matmul(
                        s_ps[:, half * ICH:(half + 1) * ICH],
                        kp[:, half, jt * 128:(jt + 1) * 128],
                        qt[:, isl],
                        start=True, stop=True,
                    )
                ex = epool.tile([128, 2 * ICH], BF16, name="ex", tag="ex")
                nc.scalar.activation(ex[:], s_ps[:], AF.Exp, scale=0.125)
                return ex

            def emit_av(r, p, jt, ex, av2):
                s = st[r]
                for half in range(2):
                    base = p * 3 * DH + half * DH
                    vl = s["v"][jt][:, base:base + 2 * DH]
                    nc.tensor.matmul(
                        av2[:, half * ICH:(half + 1) * ICH],
                        vl,
                        ex[:, half * ICH:(half + 1) * ICH],
                        start=(jt == 0), stop=(jt == NJT - 1),
                    )

            def emit_normalize(r, p, ic, av2):
                s = st[r]
                if p == 0:
                    # per-ic out rows; bufs=2 so the next ic's normalize can
                    # start while this ic's out-projection drains
                    s["aoT"] = [lpool.tile([128, ICH], BF16, name=f"aoT{c}",
                                           tag=f"aoT{c}", bufs=2)
                                for c in range(CW // 128)]
                # one fast copy releases the PSUM accumulator for the next
                # stream; the reciprocal/mul chain then runs off-critical-path
                # from the SBUF scratch
                avs = lpool.tile([128, 2 * ICH], BF16, name="avs",
                                 tag="avs", bufs=1)
                with nc.allow_low_precision(
                        reason="bf16 softmax num/den; error ~0.1% rms, "
                               "well under the 2e-2 gate"):
                    nc.vector.tensor_copy(avs[:], av2[:])
                for half in range(2):
                    l = 2 * p + half
                    ct, coff = l // 2, (l % 2) * DH
                    # even head: rows [out | sums]; odd head: [sums | out]
                    o0, s0 = (0, DH) if half == 0 else (DH, 0)
                    hsl = slice(half * ICH, (half + 1) * ICH)
                    # rc rows sit at the out-rows' base partition: the DVE
                    # mul requires equal base partitions for two SBUF inputs
                    rc = lpool.tile([128, ICH], BF16, name="rc", tag="rc",
                                    bufs=1)
                    with nc.allow_low_precision(
                            reason="bf16 softmax normalize; ~0.1% rms"):
                        nc.vector.reciprocal(rc[o0:o0 + DH, :],
                                             avs[s0:s0 + DH, hsl])
                        nc.vector.tensor_mul(
                            s["aoT"][ct][coff:coff + DH, :],
                            avs[o0:o0 + DH, hsl],
                            rc[o0:o0 + DH, :],
                        )

            def emit_ph3(r, nt, h, ao):
                s = st[r]
                po = psC.tile([128, 512], F32, name="po", tag="p1")
                lnt = nt % (ICH // 128)
                for c in range(CW // 128):
                    nc.tensor.matmul(
                        po[:],
                        ao[c][:, lnt * 128:(lnt + 1) * 128],
                        s["wo"][c][:, h * 512:(h + 1) * 512],
                        start=(c == 0), stop=(c == CW // 128 - 1),
                    )
                os_ = lpool.tile([128, 512], F32, name="os", tag="os",
                                 bufs=1)
                nc.vector.tensor_copy(os_[:], po[:])
                nc.scalar.dma_start(
                    out[nt * 128:(nt + 1) * 128, h * 512:(h + 1) * 512],
                    os_[:],
                )

            def emit_b(r, fillers):
                """Cell stream for rep r, weaving filler units into TensorE
                slack. fillers: deque of (cost_ns, emit_fn) — leftovers from
                the previous rep followed by the next rep's phase-1 units.
                This rep's ph3 groups are generated internally and take
                priority for credit pops; forced pops (stream-boundary PE
                spacers) pull phase-1 units so they never stall on the
                normalize just issued. Returns the unconsumed queue."""

                ph3q = deque()
                credit = 0.0
                cells = [(p, ic, jt) for ic in range(NIC)
                         for p in range(HPC // 2) for jt in range(NJT)]
                av_cur = None
                pend = None  # (p, ic, jt, ex, av2)

                def pop_filler(force=False):
                    nonlocal credit
                    if force:
                        q = fillers if fillers else ph3q
                    else:
                        q = ph3q if ph3q else fillers
                    if not q:
                        return False
                    cost, fn = q[0]
                    if not force and credit < cost:
                        return False
                    q.popleft()
                    fn()
                    if not force:
                        credit -= cost
                    return True

                def finish(cell):
                    nonlocal credit
                    p, ic, jt, ex, av2 = cell
                    emit_av(r, p, jt, ex, av2)
                    credit = min(credit + SLACK, CREDIT_CAP)
                    if jt == NJT - 1:
                        emit_normalize(r, p, ic, av2)
                        if p == HPC // 2 - 1:
                            for nt in range(ic * ICH // 128,
                                            (ic + 1) * ICH // 128):
                                for h in range(DM // 512):
                                    ph3q.append((600.0,
                                                 lambda nt=nt, h=h,
                                                 ao=st[r]["aoT"]:
                                                 emit_ph3(r, nt, h, ao)))
                        # PE spacer while DVE copies the accumulator out, so
                        # the next stream's first av-accumulate doesn't
                        # head-of-line stall on the "av" WAR
                        pop_filler(force=True)
                    else:
                        while pop_filler():
                            pass

                for (p, ic, jt) in cells:
                    if jt == 0:
                        av_cur = psB.tile([128, 2 * ICH], F32, name="av",
                                          tag="av")
                    ex = emit_scores(r, p, ic, jt)
                    if pend is not None:
                        finish(pend)
                    pend = (p, ic, jt, ex, av_cur)
                finish(pend)
                leftover = deque(ph3q)
                leftover.extend(fillers)
                return leftover

            # ---------------- weave reps ----------------
            boot = a_units(0)
            while boot:
                boot.popleft()[1]()  # A(0) solo
            carry = deque()
            for r in range(reps):
                fillers = deque(carry)
                if r + 1 < reps:
                    fillers.extend(a_units(r + 1))
                # rep r's tile set must exist before its cell stream starts
                while r not in st and fillers:
                    fillers.popleft()[1]()
                carry = emit_b(r, fillers)
            while carry:
                carry.popleft()[1]()

    nc.finalize()
    return nc


def make_in_maps(inputs_np):
    bf16 = mybir.dt.np(BF16)
    x = np.ascontiguousarray(inputs_np["x"], dtype=np.float32)
    w_qkv = np.asarray(inputs_np["w_qkv"], dtype=np.float32)
    w_out = np.asarray(inputs_np["w_out"], dtype=np.float32)
    in_maps = []
    xp_cache = {}
    for core in range(8):
        b, g = divmod(core, 2)
        if b not in xp_cache:
            # pack x[b]^T as [chunk, partition, k, n] so device chunk loads
            # are single linear DMAs
            xp_cache[b] = np.ascontiguousarray(
                x[b].T.reshape(KT, 128, N // NCH, NCH).transpose(2, 1, 0, 3)
            ).reshape(N // NCH, 128, KT * NCH).astype(bf16)
        xTb = xp_cache[b]
        wq = w_qkv[:, g * CW:(g + 1) * CW]
        wk = w_qkv[:, DM + g * CW:DM + (g + 1) * CW]
        wv_ = w_qkv[:, 2 * DM + g * CW:2 * DM + (g + 1) * CW]
        in_maps.append({
            "xT": xTb,
            "wqk": np.ascontiguousarray(
                np.concatenate([wq, wk], axis=1)).astype(bf16),
            "wv": np.ascontiguousarray(wv_).astype(bf16),
            "wo": np.ascontiguousarray(
                w_out[g * CW:(g + 1) * CW, :]).astype(bf16),
        })
    return in_maps


_NC_CACHE = {}


def _get_nc():
    if "nc" not in _NC_CACHE:
        _NC_CACHE["nc"] = build_nc()
    return _NC_CACHE["nc"]


def kernel(x, w_qkv, w_out, b_out):
    b_out = np.asarray(b_out, dtype=np.float32)
    nc = _get_nc()
    in_maps = make_in_maps({"x": x, "w_qkv": w_qkv, "w_out": w_out})
    res = run_bass_kernel_spmd(nc, in_maps, core_ids=list(range(8)))
    _NC_CACHE["last_result"] = res
    out = np.empty((B, N, DM), np.float32)
    for b in range(B):
        out[b] = res.results[2 * b]["out"] + res.results[2 * b + 1]["out"] + b_out
    return out
